# revision 27
# baseline (speedup 1.0000x reference)
"""LSH attention Trainium2 kernel.

Key algebraic fact (verified against the reference): the reference's
self-mask keeps dots only where query position == key position, so after
softmax the attention weights collapse to an exact one-hot and `out`
equals `v` (to ~1e-7 rel). The only real compute is the bucket hash:
    buckets[b, h*S + s] = argmax_j([r, -r])  + h*64,
    r = qk[b,s,:] @ rotations[0,:,h,:]        (32-wide per hash)

Device algorithm per (s, h) group of 32 values r:
    M  = max|r|, Mp = max(r)                (DVE grouped reduces)
    A  = BIG*|r|                            (ACT abs pass, exact: BIG=2^40)
    D  = A - iota_tbl                       (GPSIMD)     iota = 64h+32+j
    masked = BIG*M - D = BIG*(M-|r|) + iota (DVE)  == iota iff attaining
    idx = min(masked) over group            (DVE grouped reduce)
    bucket = idx - 32*(Mp == M)             (positive half attains -> -32)

Sharding: pure data parallel, one batch element per core (B=8, 8 cores).
"""

import threading

import numpy as np

B, S, D, H = 8, 4096, 64, 8
NT = 8  # s-tiles per core
TS = S // NT  # 512 s values per tile
NCH = 4  # 128-row chunks per tile
BIG = 2.0**40

_cache = {}
_lock = threading.Lock()


def _build_program(debug_taps: bool = False, reps: int = 1):
    from contextlib import ExitStack

    import concourse.bass as bass
    import concourse.tile as tile
    from concourse import bacc, mybir

    f32 = mybir.dt.float32
    i32 = mybir.dt.int32
    Alu = mybir.AluOpType
    Act = mybir.ActivationFunctionType
    X = mybir.AxisListType.X

    nc = bacc.Bacc(
        "TRN2",
        target_bir_lowering=False,
        debug=False,
        enable_asserts=False,
        num_devices=8,
    )
    qk = nc.dram_tensor("qk", [S, D], f32, kind="ExternalInput").ap()
    v = nc.dram_tensor("v", [S, D], f32, kind="ExternalInput").ap()
    rot = nc.dram_tensor("rot", [D, H * 32], f32, kind="ExternalInput").ap()
    iota = nc.dram_tensor("iota_tbl", [128, 32], f32, kind="ExternalInput").ap()
    hoff = nc.dram_tensor("hoff_tbl", [128, 32], f32, kind="ExternalInput").ap()
    ident = nc.dram_tensor("ident", [128, 128], f32, kind="ExternalInput").ap()
    out = nc.dram_tensor("out", [S, D], f32, kind="ExternalOutput").ap()
    buckets = nc.dram_tensor("buckets", [H, S], i32, kind="ExternalOutput").ap()
    if debug_taps:
        dbg_a = nc.dram_tensor("dbg_a", [128, 1024], f32, kind="ExternalOutput").ap()
        dbg_d = nc.dram_tensor("dbg_d", [128, 1024], f32, kind="ExternalOutput").ap()
        dbg_m = nc.dram_tensor("dbg_m", [128, 1024], f32, kind="ExternalOutput").ap()
        dbg_s = nc.dram_tensor("dbg_s", [128, 96], f32, kind="ExternalOutput").ap()

    with tile.TileContext(nc) as tc, ExitStack() as ctx:
        const_pool = ctx.enter_context(tc.tile_pool(name="const", bufs=1))
        qk_pool = ctx.enter_context(tc.tile_pool(name="qkp", bufs=8))
        qkt_pool = ctx.enter_context(tc.tile_pool(name="qkt", bufs=8))
        a_pool = ctx.enter_context(tc.tile_pool(name="abs", bufs=8))
        d_pool = ctx.enter_context(tc.tile_pool(name="dp", bufs=8))
        msk_pool = ctx.enter_context(tc.tile_pool(name="msk", bufs=8))
        m_pool = ctx.enter_context(tc.tile_pool(name="small", bufs=8))
        stage_pool = ctx.enter_context(tc.tile_pool(name="stage", bufs=1))
        psT_pool = ctx.enter_context(tc.tile_pool(name="psT", bufs=2, space="PSUM"))
        psR_pool = ctx.enter_context(tc.tile_pool(name="psR", bufs=2, space="PSUM"))
        psB_pool = ctx.enter_context(tc.tile_pool(name="psB", bufs=2, space="PSUM"))

        # consts on the sync HWDGE ring; PE-gating ones (ident, rot) first
        id_sb = const_pool.tile([128, 128], f32)
        nc.sync.dma_start(id_sb[:], ident)
        rot_sb = const_pool.tile([64, 256], f32)
        nc.sync.dma_start(rot_sb[:], rot)
        hoff_sb = const_pool.tile([128, 32], f32)
        nc.sync.dma_start(hoff_sb[:], hoff)
        iota_sb = const_pool.tile([128, 32], f32)
        nc.sync.dma_start(iota_sb[:], iota)
        staging = stage_pool.tile([32, NT * 128], i32)

        # qk rows (t, c, p) -> view [t][p, c, f]
        qk_v = qk.rearrange("(t c p) f -> t p c f", c=NCH, p=128)

        for _rep in range(reps):
         for t in range(NT):
            qk_sb = qk_pool.tile([128, NCH * 64], f32, tag="qk")
            nc.sync.dma_start(
                qk_sb[:].rearrange("p (c f) -> p c f", f=64), qk_v[t]
            )

            # qkT chunks via PE transpose
            psT = psT_pool.tile([64, NCH * 128], f32, tag="psT")
            for c in range(NCH):
                nc.tensor.transpose(
                    psT[:, c * 128 : (c + 1) * 128],
                    qk_sb[:, c * 64 : (c + 1) * 64],
                    id_sb[:],
                )
            qkt_sb = qkt_pool.tile([64, NCH * 128], f32, tag="qkt")
            nc.scalar.copy(qkt_sb[:], psT[:])

            # rotated chunks: psR[:, c*256:...] = qkT_c.T @ rot  -> [128 s, (8h,32j)]
            psR = psR_pool.tile([128, NCH * 256], f32, tag="psR")
            for c in range(NCH):
                nc.tensor.matmul(
                    psR[:, c * 256 : (c + 1) * 256],
                    qkt_sb[:, c * 128 : (c + 1) * 128],
                    rot_sb[:],
                )

            psR3 = psR[:].rearrange("p (g j) -> p g j", j=32)

            # A = BIG * |r|  (exact: BIG is a power of two)
            a_sb = a_pool.tile([128, NCH * 256], f32, tag="a")
            nc.scalar.activation(a_sb[:], psR[:], Act.Abs, scale=BIG)
            as3 = a_sb[:].rearrange("p (g j) -> p g j", j=32)

            # grouped reduces: M' = BIG*max|r| (SBUF), Mp = max(r) (PSUM)
            m_t = m_pool.tile([128, 32], f32, tag="m")
            nc.vector.tensor_reduce(m_t[:], as3, axis=X, op=Alu.max)
            mp_t = m_pool.tile([128, 32], f32, tag="mp")
            nc.vector.tensor_reduce(mp_t[:], psR3, axis=X, op=Alu.max)

            # G = A - M'  (exact 0 at attaining positions, <= -3e4*M elsewhere)
            d_sb = d_pool.tile([128, NCH * 256], f32, tag="d")
            mb_b = m_t[:].unsqueeze(2).broadcast_to((128, 32, 32))
            nc.gpsimd.tensor_tensor(
                d_sb[:].rearrange("p (g j) -> p g j", j=32),
                as3,
                mb_b,
                op=Alu.subtract,
            )

            # masked = G - j  (== -j at attaining positions)
            msk_sb = msk_pool.tile([128, NCH * 256], f32, tag="msk")
            iota_b = iota_sb[:].unsqueeze(1).broadcast_to((128, 32, 32))
            nc.gpsimd.tensor_tensor(
                msk_sb[:].rearrange("p (g j) -> p g j", j=32),
                d_sb[:].rearrange("p (g j) -> p g j", j=32),
                iota_b,
                op=Alu.add,
            )

            # idxneg = max(masked) = -j*  (first attaining index)
            idx_t = m_pool.tile([128, 32], f32, tag="idx")
            nc.vector.tensor_reduce(
                idx_t[:],
                msk_sb[:].rearrange("p (g j) -> p g j", j=32),
                axis=X,
                op=Alu.max,
            )

            # halfsel = (BIG*Mp == M') -> 1.0 when the positive half attains
            mpb_t = m_pool.tile([128, 32], f32, tag="mpb")
            nc.vector.tensor_scalar_mul(mpb_t[:], mp_t[:], BIG)
            hs_t = m_pool.tile([128, 32], f32, tag="hs")
            nc.vector.tensor_tensor(hs_t[:], mpb_t[:], m_t[:], op=Alu.is_equal)

            # bucket = j* + (64h+32) - 32*halfsel, written h-major (phys h*4+c)
            t1_t = m_pool.tile([128, 32], f32, tag="t1")
            nc.vector.scalar_tensor_tensor(
                t1_t[:], hs_t[:], -32.0, hoff_sb[:], op0=Alu.mult, op1=Alu.add
            )
            bf_t = m_pool.tile([128, 32], f32, tag="bf")
            bf_view = bf_t[:].rearrange("p (h c) -> p c h", c=NCH)
            t1_view = t1_t[:].rearrange("p (c h) -> p c h", h=H)
            idx_view = idx_t[:].rearrange("p (c h) -> p c h", h=H)
            nc.vector.scalar_tensor_tensor(
                bf_view, idx_view, -1.0, t1_view, op0=Alu.mult, op1=Alu.add
            )

            if debug_taps and t == 0:
                nc.sync.dma_start(dbg_a, a_sb[:])
                nc.sync.dma_start(dbg_d, d_sb[:])
                nc.sync.dma_start(dbg_m, msk_sb[:])
                nc.sync.dma_start(dbg_s[:, 0:32], m_t[:])
                nc.sync.dma_start(dbg_s[:, 32:64], mp_t[:])
                nc.sync.dma_start(dbg_s[:, 64:96], idx_t[:])

            # transpose buckets to [32=(h,c), 128=p] and convert to int32
            psB = psB_pool.tile([32, 128], f32, tag="psB")
            nc.tensor.transpose(psB[:], bf_t[:], id_sb[:])
            nc.vector.tensor_copy(staging[:, t * 128 : (t + 1) * 128], psB[:])

        # out = v: big copy issued last (ACT HWDGE ring), overlaps compute
        nc.scalar.dma_start(out, v)

        # buckets[h, t*512 + c*128 + p] <- staging[h*4+c, t*128+p]
        bk_v = buckets.rearrange("h (t c p) -> h c t p", c=NCH, p=128)
        for h in range(H):
            nc.sync.dma_start(
                bk_v[h],
                staging[h * NCH : (h + 1) * NCH, :].rearrange(
                    "c (t p) -> c t p", p=128
                ),
            )

    nc.compile()
    return nc


def _make_consts():
    # iota_tbl[p, j] = -j   (negated index for max-extraction; bcast over groups)
    jj = np.arange(32, dtype=np.float32)
    tbl = np.tile(-jj, (128, 1)).astype(np.float32)  # [128, 32]
    # hoff_tbl[p, c*8 + h] = 64*h + 32
    hh = 64.0 * np.arange(H, dtype=np.float32) + 32.0
    hofft = np.tile(np.tile(hh, NCH), (128, 1)).astype(np.float32)  # [128, 32]
    ident = np.eye(128, dtype=np.float32)
    return tbl, hofft, ident


def kernel(qk: np.ndarray, v: np.ndarray, rotations: np.ndarray):
    import os

    from concourse.bass_utils import run_bass_kernel_spmd

    with _lock:
        if "nc" not in _cache:
            _cache["nc"] = _build_program()
    nc = _cache["nc"]
    trace = bool(int(os.environ.get("KERNEL_TRACE", "0")))

    rot2 = np.ascontiguousarray(rotations.reshape(D, H * 32)).astype(np.float32)
    tbl, hofft, ident = _make_consts()
    in_maps = []
    for b in range(B):
        in_maps.append(
            {
                "qk": np.ascontiguousarray(qk[b]),
                "v": np.ascontiguousarray(v[b]),
                "rot": rot2,
                "iota_tbl": tbl,
                "hoff_tbl": hofft,
                "ident": ident,
            }
        )
    res = run_bass_kernel_spmd(nc, in_maps, core_ids=list(range(B)), trace=trace)
    _cache["last_res"] = res
    out = np.stack([r["out"] for r in res.results]).astype(np.float32)
    buckets = np.stack(
        [r["buckets"].reshape(-1) for r in res.results]
    ).astype(np.int32)
    return out, buckets
